# revision 1
# baseline (speedup 1.0000x reference)
"""Trainium2 Bass kernel for GazeKLDUnit loss.

reference:
    pred_means = pred[:, :2]              # [B, 2]
    true_means = true.mean(axis=1)        # [B, 2]  (mean over T=50)
    kld = 0.5 * sum((true_means - pred_means)**2, -1)   # [B]
    out = mean(kld)                       # scalar

Strategy: data-parallel over 8 cores (B/8 rows each). Each core streams its
52.4MB shard of `true` through SBUF in [128, 3200] tiles (1.6MB per DMA,
contiguous 12.8KB per partition), reduces the T axis on the vector engine
with a strided access pattern, subtracts host-prescaled pred (50*pred so the
T-mean never has to be materialized), squares and row-reduces to a [128, 1]
partial per core. Host sums the 8*128 partials in float64 and applies the
0.5 / (T^2 * B) scale.
"""

from contextlib import ExitStack

import numpy as np

import concourse.bass as bass
import concourse.mybir as mybir
import concourse.tile as tile
from concourse.bass_utils import run_bass_kernel_spmd

N_CORES = 8
B = 1048576
T = 50
ROW = 2 * T                # 100 floats per row of `true`
BS = B // N_CORES          # 131072 rows per core
K = 32                     # rows of `true` per partition per tile
TILE_ROWS = 128 * K        # 4096 rows per tile
N_TILES = BS // TILE_ROWS  # 32 tiles per core
F = K * ROW                # 3200 floats per partition per tile
MW = K * 2                 # 64 sums per partition per tile
W = N_TILES * MW           # 2048 sums per partition per core

_nc_cache = {}


NBUF = 4


def _build(dtype=mybir.dt.float32):
    nc = bass.Bass()
    t_in = nc.dram_tensor("t", [N_TILES, 128, F], dtype, kind="ExternalInput")
    p_in = nc.dram_tensor("p", [128, W], dtype, kind="ExternalInput")
    o_out = nc.dram_tensor("o", [128, 1], dtype, kind="ExternalOutput")

    n_dma = N_TILES + 2  # pred + stream tiles + output store

    with (
        nc.Block() as block,
        nc.semaphore("dma_sem") as dma_sem,
        nc.semaphore("vec_sem") as vec_sem,
        nc.sbuf_tensor("tt", [128, NBUF * F], dtype) as tt,
        nc.sbuf_tensor("pred_t", [128, W], dtype) as pred_t,
        nc.sbuf_tensor("msum", [128, W], dtype) as msum,
        nc.sbuf_tensor("dbuf", [128, W], dtype) as dbuf,
        nc.sbuf_tensor("d2buf", [128, W], dtype) as d2buf,
        nc.sbuf_tensor("accb", [128, 1], dtype) as accb,
    ):

        @block.sync
        def _(sync):
            sync.dma_start(pred_t[:, :], p_in[:, :]).then_inc(dma_sem, 16)
            for i in range(N_TILES):
                slot = i % NBUF
                if i >= NBUF:
                    # slot reuse: previous occupant's reduce must be retired
                    sync.wait_ge(vec_sem, i - NBUF + 1)
                sync.dma_start(
                    tt[:, slot * F : (slot + 1) * F], t_in[i]
                ).then_inc(dma_sem, 16)
            sync.wait_ge(vec_sem, N_TILES + 2)
            sync.dma_start(o_out[:, :], accb[:, :]).then_inc(dma_sem, 16)
            sync.wait_ge(dma_sem, 16 * n_dma)

        @block.vector
        def _(vector):
            for i in range(N_TILES):
                vector.wait_ge(dma_sem, 16 * (i + 2))  # pred + tiles 0..i
                slot = i % NBUF
                v = tt[:, slot * F : (slot + 1) * F].rearrange(
                    "p (r t c) -> p r c t", r=K, t=T, c=2
                )
                vector.reduce_sum(
                    msum[:, i * MW : (i + 1) * MW], v, axis=mybir.AxisListType.X
                ).then_inc(vec_sem, 1)
            vector.tensor_sub(dbuf[:, :], msum[:, :], pred_t[:, :]).then_inc(
                vec_sem, 1
            )
            vector.tensor_mul(d2buf[:, :], dbuf[:, :], dbuf[:, :])
            vector.reduce_sum(
                accb[:, :], d2buf[:, :], axis=mybir.AxisListType.X
            ).then_inc(vec_sem, 1)

    return nc


def _strip_redundant_dma_waits(nc):
    """Drop transitively-redundant waits from DMACopy instructions.

    The DMA ISA slot has exactly one sync-wait. Tile's scheduler emits both a
    WAR wait (consumer engine sem) and a WAW wait (previous DMA's lane sem) on
    slot-reusing stream DMAs, and its wait minimization is not transitive
    across processors, so the WAW wait survives even though the consumer
    already observed the previous DMA's completion. Walrus then refuses to
    codegen the 2-wait DMA. This pass removes any DMA wait that is implied by
    another wait on a sem owned by a single in-order compute engine whose
    instruction stream already waited on the dropped sem at an equal-or-higher
    value (semaphores are monotonic, so an earlier-held wait still holds).
    """
    for func in nc.m.functions:
        insts = [i for b in func.blocks for i in b.instructions]
        per_engine = {}
        for inst in insts:
            per_engine.setdefault(inst.engine, []).append(inst)

        is_dma = lambda i: type(i).__name__ in ("InstDMACopy", "InstDrain")
        # sem id -> engine that exclusively updates it via non-DMA instructions
        sem_owner = {}
        sem_bad = set()
        for inst in insts:
            si = inst.sync_info
            if not si or not si.on_update:
                continue
            for u in si.on_update:
                if is_dma(inst) or getattr(u, "update_reg", None) is not None:
                    sem_bad.add(u.id)
                elif sem_owner.setdefault(u.id, inst.engine) != inst.engine:
                    sem_bad.add(u.id)

        # per engine: cumulative updates per sem and running max wait per sem
        eng_info = {}
        for eng, elist in per_engine.items():
            cum = {}
            maxw = {}
            steps = []  # per instr: (cum_after copy per-sem touched, maxw snapshot)
            for inst in elist:
                si = inst.sync_info
                if si and si.on_wait:
                    for w in si.on_wait:
                        if w.wait_mode == "sem-ge-imm" and w.wait_value is not None:
                            maxw[w.id] = max(maxw.get(w.id, 0), w.wait_value)
                if si and si.on_update and not is_dma(inst):
                    for u in si.on_update:
                        if u.update_mode == "sem-add-imm" and u.update_value:
                            cum[u.id] = cum.get(u.id, 0) + u.update_value
                steps.append((dict(cum), dict(maxw)))
            eng_info[eng] = (elist, steps)

        def implied(src_wait, tgt_wait):
            """True if src (s1>=v1) being satisfied implies tgt (s2>=v2)."""
            s1, v1 = src_wait.id, src_wait.wait_value
            s2, v2 = tgt_wait.id, tgt_wait.wait_value
            if src_wait.wait_mode != "sem-ge-imm" or tgt_wait.wait_mode != "sem-ge-imm":
                return False
            if s1 in sem_bad or s1 not in sem_owner:
                return False
            eng = sem_owner[s1]
            elist, steps = eng_info[eng]
            # find last engine instr whose cumulative s1 update is <= v1:
            # s1 >= v1 guarantees that instr (and all before it) completed.
            jstar = -1
            for j, (cum, _) in enumerate(steps):
                if cum.get(s1, 0) <= v1:
                    jstar = j
                else:
                    break
            if jstar < 0:
                return False
            cum, maxw = steps[jstar]
            return maxw.get(s2, 0) >= v2 or cum.get(s2, 0) >= v2

        for inst in insts:
            if type(inst).__name__ != "InstDMACopy":
                continue
            si = inst.sync_info
            if not si or not si.on_wait or len(si.on_wait) <= 1:
                continue
            waits = list(si.on_wait)
            kept = []
            for k, w in enumerate(waits):
                others = kept + [x for x in waits[k + 1 :]]
                if any(implied(o, w) for o in others):
                    continue
                kept.append(w)
            if len(kept) < len(waits):
                si.on_wait = kept
            assert len(kept) <= 1, (
                f"{inst.name}: still {len(kept)} waits after stripping: "
                f"{[str(w) for w in kept]}"
            )


def _prep_inputs(pred, true):
    """Build per-core input maps (host-side shard + pack)."""
    true_flat = np.ascontiguousarray(true).reshape(B, ROW)
    pred50 = np.ascontiguousarray(pred[:, :2]) * np.float32(T)  # [B, 2]
    in_maps = []
    for c in range(N_CORES):
        t_shard = true_flat[c * BS : (c + 1) * BS].reshape(N_TILES, 128, F)
        p_shard = (
            pred50[c * BS : (c + 1) * BS]
            .reshape(N_TILES, 128, K, 2)
            .transpose(1, 0, 2, 3)
            .reshape(128, W)
        )
        in_maps.append(
            {"t": np.ascontiguousarray(t_shard), "p": np.ascontiguousarray(p_shard)}
        )
    return in_maps


def _finish(results):
    total = np.float64(0.0)
    for r in results:
        total += r["o"].astype(np.float64).sum()
    val = total * 0.5 / (T * T) / B
    return np.array(val, dtype=np.float32)


def _get_nc():
    if "nc" not in _nc_cache:
        _nc_cache["nc"] = _build()
    return _nc_cache["nc"]


def kernel(pred, true):
    nc = _get_nc()
    in_maps = _prep_inputs(pred, true)
    res = run_bass_kernel_spmd(nc, in_maps, list(range(N_CORES)))
    return _finish(res.results)


def kernel_traced(pred, true, **trace_kwargs):
    nc = _get_nc()
    in_maps = _prep_inputs(pred, true)
    res = run_bass_kernel_spmd(
        nc, in_maps, list(range(N_CORES)), trace=True, **trace_kwargs
    )
    return _finish(res.results), res



# revision 3
# speedup vs baseline: 1.1312x; 1.1312x over previous
"""Trainium2 Bass kernel for GazeKLDUnit loss.

reference:
    pred_means = pred[:, :2]              # [B, 2]
    true_means = true.mean(axis=1)        # [B, 2]  (mean over T=50)
    kld = 0.5 * sum((true_means - pred_means)**2, -1)   # [B]
    out = mean(kld)                       # scalar

Strategy: data-parallel over 8 cores (B/8 rows each). The problem is pure
HBM-bandwidth: every element of `true` is touched once. Host-side we cast to
bf16 (quantization error ~4e-7 on the final scalar, gate is 2e-2) and
de-interleave each row's [T, 2] block to [2, T] so the T-reduction runs on
contiguous 50-element segments — that makes the DVE reduce eligible for the
packed 2x_1p perf mode AND halves the bytes DMA'd from HBM. Each core streams
its 26.2MB bf16 shard through SBUF in [128, 6400] tiles (1.64MB per DMA,
12.8KB contiguous per partition), reduces T on the vector engine, subtracts
host-prescaled pred (T*pred so the mean never has to be materialized),
squares and row-reduces to a [128, 1] f32 partial per core. Host sums the
8*128 partials in float64 and applies the 0.5 / (T^2 * B) scale.
"""

import ml_dtypes
import numpy as np

import concourse.bass as bass
import concourse.mybir as mybir
from concourse.bass_utils import run_bass_kernel_spmd

BF16 = ml_dtypes.bfloat16

N_CORES = 8
B = 1048576
T = 50
BS = B // N_CORES          # 131072 rows per core
K = 64                     # rows of `true` per partition per tile
TILE_ROWS = 128 * K        # 8192 rows per tile
N_TILES = BS // TILE_ROWS  # 16 tiles per core
F = K * 2 * T              # 6400 bf16 per partition per tile (12.8KB)
MW = K * 2                 # 128 sums per partition per tile
W = N_TILES * MW           # 2048 sums per partition per core

NBUF = 4

_nc_cache = {}


def _build():
    bf = mybir.dt.bfloat16
    f32 = mybir.dt.float32
    nc = bass.Bass()
    t_in = nc.dram_tensor("t", [N_TILES, 128, F], bf, kind="ExternalInput")
    p_in = nc.dram_tensor("p", [128, W], bf, kind="ExternalInput")
    o_out = nc.dram_tensor("o", [128, 1], f32, kind="ExternalOutput")

    n_dma = N_TILES + 2  # pred + stream tiles + output store

    with (
        nc.allow_low_precision(
            reason="bf16 partials; final scalar mean tolerates ~1e-4"
        ),
        nc.Block() as block,
        nc.semaphore("dma_sem") as dma_sem,
        nc.semaphore("vec_sem") as vec_sem,
        nc.sbuf_tensor("tt", [128, NBUF * F], bf) as tt,
        nc.sbuf_tensor("pred_t", [128, W], bf) as pred_t,
        nc.sbuf_tensor("msum", [128, W], bf) as msum,
        nc.sbuf_tensor("dbuf", [128, W], bf) as dbuf,
        nc.sbuf_tensor("d2buf", [128, W], bf) as d2buf,
        nc.sbuf_tensor("accb", [128, 1], f32) as accb,
    ):

        @block.sync
        def _(sync):
            sync.dma_start(pred_t[:, :], p_in[:, :]).then_inc(dma_sem, 16)
            for i in range(N_TILES):
                slot = i % NBUF
                if i >= NBUF:
                    # slot reuse: previous occupant's reduce must be retired
                    sync.wait_ge(vec_sem, i - NBUF + 1)
                sync.dma_start(
                    tt[:, slot * F : (slot + 1) * F], t_in[i]
                ).then_inc(dma_sem, 16)
            sync.wait_ge(vec_sem, N_TILES + 2)
            sync.dma_start(o_out[:, :], accb[:, :]).then_inc(dma_sem, 16)
            sync.wait_ge(dma_sem, 16 * n_dma)

        @block.vector
        def _(vector):
            for i in range(N_TILES):
                vector.wait_ge(dma_sem, 16 * (i + 2))  # pred + tiles 0..i
                slot = i % NBUF
                v = tt[:, slot * F : (slot + 1) * F].rearrange(
                    "p (s t) -> p s t", s=MW, t=T
                )
                vector.reduce_sum(
                    msum[:, i * MW : (i + 1) * MW], v, axis=mybir.AxisListType.X
                ).then_inc(vec_sem, 1)
            vector.tensor_sub(dbuf[:, :], msum[:, :], pred_t[:, :]).then_inc(
                vec_sem, 1
            )
            vector.tensor_mul(d2buf[:, :], dbuf[:, :], dbuf[:, :])
            vector.reduce_sum(
                accb[:, :], d2buf[:, :], axis=mybir.AxisListType.X
            ).then_inc(vec_sem, 1)

    return nc


def _prep_inputs(pred, true):
    """Build per-core input maps (host-side shard + cast + de-interleave)."""
    true_bf = np.asarray(true).astype(BF16)                      # [B, T, 2]
    pred50 = (np.asarray(pred[:, :2]) * np.float32(T)).astype(BF16)  # [B, 2]
    in_maps = []
    for c in range(N_CORES):
        sh = true_bf[c * BS : (c + 1) * BS]                      # [BS, T, 2]
        t_shard = np.ascontiguousarray(sh.transpose(0, 2, 1)).reshape(
            N_TILES, 128, F
        )
        p_shard = (
            pred50[c * BS : (c + 1) * BS]
            .reshape(N_TILES, 128, K, 2)
            .transpose(1, 0, 2, 3)
            .reshape(128, W)
        )
        in_maps.append({"t": t_shard, "p": np.ascontiguousarray(p_shard)})
    return in_maps


def _finish(results):
    total = np.float64(0.0)
    for r in results:
        total += r["o"].astype(np.float64).sum()
    val = total * 0.5 / (T * T) / B
    return np.array(val, dtype=np.float32)


def _get_nc():
    if "nc" not in _nc_cache:
        _nc_cache["nc"] = _build()
    return _nc_cache["nc"]


def kernel(pred, true):
    nc = _get_nc()
    in_maps = _prep_inputs(pred, true)
    res = run_bass_kernel_spmd(nc, in_maps, list(range(N_CORES)))
    return _finish(res.results)


def kernel_traced(pred, true, **trace_kwargs):
    nc = _get_nc()
    in_maps = _prep_inputs(pred, true)
    res = run_bass_kernel_spmd(
        nc, in_maps, list(range(N_CORES)), trace=True, **trace_kwargs
    )
    return _finish(res.results), res


# revision 5
# speedup vs baseline: 1.4159x; 1.2517x over previous
"""Trainium2 Bass kernel for GazeKLDUnit loss.

reference:
    pred_means = pred[:, :2]              # [B, 2]
    true_means = true.mean(axis=1)        # [B, 2]  (mean over T=50)
    kld = 0.5 * sum((true_means - pred_means)**2, -1)   # [B]
    out = mean(kld)                       # scalar

Strategy: data-parallel over 8 cores (B/8 rows each). The problem is pure
HBM bandwidth, so bytes are the lever: host casts to bf16 (final-scalar
quantization error ~4e-7, gate 2e-2), halving DMA. Measured on HW, the DVE
tensor-reduce runs at 1 elem/cycle/lane regardless of dtype/layout, but
tensor_tensor runs at 2 elem/cycle on packed bf16 — so each row's T=50
samples are packed host-side as [t2=2, k, c, t0=25] and the kernel first
folds the two t2 halves with one contiguous tensor_add (2x mode), then
reduces the remaining 25-element segments (1x). That cuts vector time per
tile from 6.7us to ~5us, just under the 4.8us DMA stream time per tile.
Each core streams 16 [128, 6400] bf16 tiles (1.64MB per DMA), subtracts
host-prescaled pred (T*pred), squares and row-reduces to a [128, 1] f32
partial. Host sums partials in float64 and applies 0.5 / (T^2 * B).
"""

import ml_dtypes
import numpy as np

import concourse.bass as bass
import concourse.mybir as mybir
from concourse.bass_utils import run_bass_kernel_spmd

BF16 = ml_dtypes.bfloat16

N_CORES = 8
B = 1048576
T = 50
TH = T // 2                # 25: reduced segment length after the fold
BS = B // N_CORES          # 131072 rows per core
K = 64                     # rows of `true` per partition per tile
TILE_ROWS = 128 * K        # 8192 rows per tile
N_TILES = BS // TILE_ROWS  # 16 tiles per core
F = K * 2 * T              # 6400 bf16 per partition per tile (12.8KB)
FH = F // 2                # 3200: folded tile size
MW = K * 2                 # 128 sums per partition per tile
W = N_TILES * MW           # 2048 sums per partition per core

NBUF = 6

_nc_cache = {}


def _build():
    bf = mybir.dt.bfloat16
    f32 = mybir.dt.float32
    nc = bass.Bass()
    t_in = nc.dram_tensor("t", [N_TILES, 128, F], bf, kind="ExternalInput")
    p_in = nc.dram_tensor("p", [128, W], bf, kind="ExternalInput")
    o_out = nc.dram_tensor("o", [128, 1], f32, kind="ExternalOutput")

    n_dma = N_TILES + 2  # stream tiles + pred + output store

    with (
        nc.allow_low_precision(
            reason="bf16 partials; final scalar mean tolerates ~1e-4"
        ),
        nc.Block() as block,
        nc.semaphore("dma_sem") as dma_sem,
        nc.semaphore("vec_sem") as vec_sem,
        nc.sbuf_tensor("tt", [128, NBUF * F], bf) as tt,
        nc.sbuf_tensor("fold", [128, FH], bf) as fold,
        nc.sbuf_tensor("pred_t", [128, W], bf) as pred_t,
        nc.sbuf_tensor("msum", [128, W], bf) as msum,
        nc.sbuf_tensor("dbuf", [128, W], bf) as dbuf,
        nc.sbuf_tensor("d2buf", [128, W], bf) as d2buf,
        nc.sbuf_tensor("accb", [128, 1], f32) as accb,
    ):

        @block.sync
        def _(sync):
            for i in range(N_TILES):
                slot = i % NBUF
                if i >= NBUF:
                    # slot reuse: previous occupant's fold must have consumed it
                    sync.wait_ge(vec_sem, i - NBUF + 1)
                sync.dma_start(
                    tt[:, slot * F : (slot + 1) * F], t_in[i]
                ).then_inc(dma_sem, 16)
            sync.dma_start(pred_t[:, :], p_in[:, :]).then_inc(dma_sem, 16)
            sync.wait_ge(vec_sem, N_TILES + 2)
            sync.dma_start(o_out[:, :], accb[:, :]).then_inc(dma_sem, 16)
            sync.wait_ge(dma_sem, 16 * n_dma)

        @block.vector
        def _(vector):
            for i in range(N_TILES):
                vector.wait_ge(dma_sem, 16 * (i + 1))
                slot = i % NBUF
                a = tt[:, slot * F : slot * F + FH]
                b = tt[:, slot * F + FH : (slot + 1) * F]
                # 2x-mode contiguous fold of the two t2 halves
                vector.tensor_add(fold[:, :], a, b).then_inc(vec_sem, 1)
                v = fold[:, :].rearrange("p (s t) -> p s t", s=MW, t=TH)
                vector.reduce_sum(
                    msum[:, i * MW : (i + 1) * MW], v, axis=mybir.AxisListType.X
                )
            vector.wait_ge(dma_sem, 16 * (N_TILES + 1))  # pred loaded
            vector.tensor_sub(dbuf[:, :], msum[:, :], pred_t[:, :]).then_inc(
                vec_sem, 1
            )
            vector.tensor_mul(d2buf[:, :], dbuf[:, :], dbuf[:, :])
            vector.reduce_sum(
                accb[:, :], d2buf[:, :], axis=mybir.AxisListType.X
            ).then_inc(vec_sem, 1)

    return nc


def _prep_inputs(pred, true):
    """Per-core input maps: shard + cast bf16 + [t2, k, c, t0] pack."""
    true_bf = np.asarray(true).astype(BF16)                      # [B, T, 2]
    pred50 = (np.asarray(pred[:, :2]) * np.float32(T)).astype(BF16)  # [B, 2]
    in_maps = []
    for c in range(N_CORES):
        sh = true_bf[c * BS : (c + 1) * BS]                      # [BS, T, 2]
        # [i, p, k, t2, t0, c] -> [i, p, t2, k, c, t0]
        t_shard = np.ascontiguousarray(
            sh.reshape(N_TILES, 128, K, 2, TH, 2).transpose(0, 1, 3, 2, 5, 4)
        ).reshape(N_TILES, 128, F)
        p_shard = (
            pred50[c * BS : (c + 1) * BS]
            .reshape(N_TILES, 128, K, 2)
            .transpose(1, 0, 2, 3)
            .reshape(128, W)
        )
        in_maps.append({"t": t_shard, "p": np.ascontiguousarray(p_shard)})
    return in_maps


def _finish(results):
    total = np.float64(0.0)
    for r in results:
        total += r["o"].astype(np.float64).sum()
    val = total * 0.5 / (T * T) / B
    return np.array(val, dtype=np.float32)


def _get_nc():
    if "nc" not in _nc_cache:
        _nc_cache["nc"] = _build()
    return _nc_cache["nc"]


def kernel(pred, true):
    nc = _get_nc()
    in_maps = _prep_inputs(pred, true)
    res = run_bass_kernel_spmd(nc, in_maps, list(range(N_CORES)))
    return _finish(res.results)


def kernel_traced(pred, true, **trace_kwargs):
    nc = _get_nc()
    in_maps = _prep_inputs(pred, true)
    res = run_bass_kernel_spmd(
        nc, in_maps, list(range(N_CORES)), trace=True, **trace_kwargs
    )
    return _finish(res.results), res
